# revision 1
# baseline (speedup 1.0000x reference)
"""Trainium2 Bass kernel for nn_AdvancedLiquidNeuralNetwork.

Data-parallel over 8 NeuronCores (batch sharded). Per core, batch-major
compute pipeline with a feature-major f32r h-state:

  xin = x @ W_in.T                  (PE, x.T-stationary -> batch-major PSUM)
  t=0:  h1 = c * tanh(xin + liq0c)  (liq0c host-derived; h kept transposed)
  t>=1: y_l = h @ (W_liq_l+I).T     (PE, hT-stationary, 3 layers fused in N)
        LN_l via bn_stats/bn_aggr + bit-seed Babylonian sqrt + fused merges
        att  = liq @ (W_o W_v + I).T (PE, liqT-stationary)
        z    = LN(att) + xin        (affine_then_add), t = tanh(z)
        h'T  = a*hT + c*tT          (feature-major update)
  out = relu([h3 h0] @ W1.T + b1) @ W2.T + b2   (feature-major, fused ACT)

Heavy matmuls in float32r (tf32-class, ~1.6e-4 rel err).
"""
import sys
import numpy as np
from contextlib import ExitStack

sys.path.insert(0, "/opt/trn_rl_repo")

import concourse.bacc as bacc
import concourse.bass as bass
import concourse.tile as tile
from concourse import mybir
from concourse.bass_utils import run_bass_kernel_spmd
from concourse.masks import make_identity

F32 = mybir.dt.float32
F32R = mybir.dt.float32r
U32 = mybir.dt.uint32
AF = mybir.ActivationFunctionType
OP = mybir.AluOpType

LN_EPS = 1e-5
TAU_EPS = 1e-6
N_CORES = 8
P = 128


def _np_ln(x, eps=LN_EPS):
    m = x.mean(-1, keepdims=True)
    v = x.var(-1, keepdims=True)
    return (x - m) / np.sqrt(v + eps)


def build_nc(B_shard, H, IN, OUT, L, steps, CHUNK, host):
    nc = bacc.Bacc("TRN2")
    n_chunks = B_shard // CHUNK
    TPC = CHUNK // P       # tiles per chunk
    KH = H // P            # 4
    KI = IN // P           # 2
    NL = L                 # liquid moving n-chunks (each H wide)
    M1 = H // P            # 4 m-tiles of out1
    K1 = 2 * H // P        # 8 k-tiles of out1
    c_imm, a_imm = host["c_imm"], host["a_imm"]

    xT = nc.dram_tensor("xT", [IN, B_shard], F32, kind="ExternalInput")
    wliq = nc.dram_tensor("wliq", [H, L * H], F32, kind="ExternalInput")
    watt = nc.dram_tensor("watt", [H, H], F32, kind="ExternalInput")
    winT = nc.dram_tensor("winT", [IN, H], F32, kind="ExternalInput")
    w1T = nc.dram_tensor("w1T", [2 * H, H], F32, kind="ExternalInput")
    w2T = nc.dram_tensor("w2T", [H, OUT], F32, kind="ExternalInput")
    rows = nc.dram_tensor("rows", [8, H], F32, kind="ExternalInput")
    bliqrow = nc.dram_tensor("bliqrow", [1, L * H], F32, kind="ExternalInput")
    battrow = nc.dram_tensor("battrow", [1, H], F32, kind="ExternalInput")
    b1row = nc.dram_tensor("b1row", [1, H], F32, kind="ExternalInput")
    b2row = nc.dram_tensor("b2row", [1, OUT], F32, kind="ExternalInput")
    out_t = nc.dram_tensor("out_t", [OUT, B_shard], F32, kind="ExternalOutput")

    with tile.TileContext(nc) as tc, ExitStack() as ctx:
        wpool = ctx.enter_context(tc.tile_pool(name="wpool", bufs=1))
        stage = ctx.enter_context(tc.tile_pool(name="stage", bufs=1))
        state = ctx.enter_context(tc.tile_pool(name="state", bufs=1))
        work = ctx.enter_context(tc.tile_pool(name="work", bufs=2))
        stats = ctx.enter_context(tc.tile_pool(name="stats", bufs=3))
        psY = ctx.enter_context(tc.tile_pool(name="psY", bufs=2, space="PSUM"))
        psA = ctx.enter_context(tc.tile_pool(name="psA", bufs=1, space="PSUM"))
        psT = ctx.enter_context(tc.tile_pool(name="psT", bufs=1, space="PSUM"))

        def load_round(dram, nrow, ncol, name):
            outs = []
            for k in range(nrow // P):
                st = stage.tile([P, ncol], F32, name=f"st_{name}{k}", tag="stage")
                t = wpool.tile([P, ncol], F32R, name=f"{name}{k}")
                nc.sync.dma_start(out=st, in_=dram[k * P:(k + 1) * P, :])
                nc.scalar.activation(t, st, AF.Copy)
                outs.append(t)
            return outs

        wliq_sb = load_round(wliq, H, L * H, "wliq")
        watt_sb = load_round(watt, H, H, "watt")
        winT_sb = load_round(winT, IN, H, "winT")
        w1T_sb = load_round(w1T, 2 * H, H, "w1T")
        w2T_sb = load_round(w2T, H, OUT, "w2T")

        ident = wpool.tile([P, P], F32, name="ident")
        make_identity(nc, ident)
        identr = wpool.tile([P, P], F32R, name="identr")
        nc.scalar.activation(identr, ident, AF.Copy)

        def bcast_row(row_ap, n, name):
            t = wpool.tile([P, n], F32, name=name)
            src = bass.AP(tensor=row_ap.tensor, offset=row_ap.offset, ap=[[0, P], [1, n]])
            nc.sync.dma_start(out=t, in_=src)
            return t

        liq0c_b = None if host["liq0c_zero"] else bcast_row(rows[0:1, :], H, "liq0c_b")
        xrow_b = None if host["xrow_zero"] else bcast_row(rows[1:2, :], H, "xrow_b")
        g_b = None if host["g_trivial"] else [bcast_row(rows[4 + l:5 + l, :], H, f"g_b{l}") for l in range(L)]
        gatt_b = None if host["gatt_trivial"] else bcast_row(rows[7:8, :], H, "gatt_b")
        bliq_b = None if host["bliq_zero"] else bcast_row(bliqrow[0:1, :], L * H, "bliq_b")
        batt_b = None if host["batt_zero"] else bcast_row(battrow[0:1, :], H, "batt_b")
        # per-k-block per-partition vectors (feature-major): [P, KH] column k
        cvec_sb = avec_sb = cvec_row_b = None
        if c_imm is None:
            cvec_row_b = bcast_row(rows[2:3, :], H, "cvec_row_b")
            cvec_sb = wpool.tile([P, KH], F32, name="cvec_sb")
            nc.sync.dma_start(out=cvec_sb, in_=bass.AP(
                tensor=rows.tensor, offset=rows[2:3, :].offset, ap=[[1, P], [P, KH]]))
            avec_sb = wpool.tile([P, KH], F32, name="avec_sb")
            nc.sync.dma_start(out=avec_sb, in_=bass.AP(
                tensor=rows.tensor, offset=rows[3:4, :].offset, ap=[[1, P], [P, KH]]))
        b1_sb = None
        if not host["b1_zero"]:
            b1_sb = wpool.tile([P, M1], F32, name="b1_sb")
            nc.sync.dma_start(out=b1_sb, in_=bass.AP(
                tensor=b1row.tensor, offset=0, ap=[[1, P], [P, M1]]))
        b2_sb = None
        if not host["b2_zero"]:
            b2_sb = wpool.tile([P, 1], F32, name="b2_sb")
            nc.sync.dma_start(out=b2_sb, in_=bass.AP(
                tensor=b2row.tensor, offset=0, ap=[[1, P], [P, 1]]))

        def transpose_to(dst, src_bm):
            """dst [P, KH*P] f32r <- blockwise transpose of batch-major [P, H] f32r."""
            tp = psT.tile([P, H], F32R, name="tp", tag="psT")
            for k in range(KH):
                nc.tensor.transpose(tp[:, k * P:(k + 1) * P], src_bm[:, k * P:(k + 1) * P], identr)
            nc.scalar.activation(dst, tp, AF.Copy)

        # const tiles for the all-GpSimd rsqrt Newton chain
        rsqC = wpool.tile([P, 8], U32, name="rsqC")
        nc.vector.memset(rsqC, 0xBE6EB3BE)
        onesC = wpool.tile([P, 8], U32, name="onesC")
        nc.vector.memset(onesC, 1)
        epsC = wpool.tile([P, 8], F32, name="epsC")
        nc.vector.memset(epsC, LN_EPS)
        nhalfC = wpool.tile([P, 8], F32, name="nhalfC")
        nc.vector.memset(nhalfC, -0.5)
        c15C = wpool.tile([P, 8], F32, name="c15C")
        nc.vector.memset(c15C, 1.5)
        negC = wpool.tile([P, 8], F32, name="negC")
        nc.vector.memset(negC, -1.0)

        def rstd_chain(mv, G, name):
            """mv [P, 2, G] (mean;var rows) -> R (=1/std) [P,G], MB (=-mean/std)."""
            M = mv[:, 0, :]
            V = mv[:, 1, :]
            w = stats.tile([P, G], F32, name=f"w_{name}", tag=f"w{G}")
            nc.vector.tensor_scalar_add(out=w, in0=V, scalar1=LN_EPS)
            R = stats.tile([P, G], F32, name=f"R_{name}", tag=f"R{G}")
            nc.vector.tensor_tensor(out=R.bitcast(U32), in0=rsqC[:, 0:G],
                                    in1=w.bitcast(U32), op=OP.subtract)
            nc.vector.tensor_scalar(out=R.bitcast(U32), in0=R.bitcast(U32),
                                    scalar1=1, scalar2=None, op0=OP.logical_shift_right)
            w2n = stats.tile([P, G], F32, name=f"w2n_{name}", tag=f"w2n{G}")
            nc.vector.tensor_scalar(out=w2n, in0=w, scalar1=-0.5, scalar2=None, op0=OP.mult)
            aa = stats.tile([P, G], F32, name=f"aa_{name}", tag=f"aa{G}")
            for _ in range(2):
                nc.vector.scalar_tensor_tensor(out=aa, in0=R, scalar=1.0, in1=R,
                                               op0=OP.mult, op1=OP.mult)
                nc.vector.tensor_tensor(out=aa, in0=aa, in1=w2n, op=OP.mult)
                nc.vector.scalar_tensor_tensor(out=R, in0=aa, scalar=1.5, in1=R,
                                               op0=OP.add, op1=OP.mult)
            MB = stats.tile([P, G], F32, name=f"MB_{name}", tag=f"MB{G}")
            nc.vector.scalar_tensor_tensor(out=MB, in0=M, scalar=-1.0, in1=R,
                                           op0=OP.mult, op1=OP.mult)
            return R, MB

        for ci in range(n_chunks):
            c0 = ci * CHUNK
            xT_sb = []
            for k in range(KI):
                xst = stage.tile([P, CHUNK], F32, name=f"xst{ci}_{k}", tag="xstage", bufs=2)
                xsb = state.tile([P, CHUNK], F32R, name=f"xTs{ci}_{k}", tag=f"xT{k}", bufs=2)
                nc.sync.dma_start(out=xst, in_=xT[k * P:(k + 1) * P, c0:c0 + CHUNK])
                nc.scalar.activation(xsb, xst, AF.Copy)
                xT_sb.append(xsb)

            xinb_t = [state.tile([P, H], F32, name=f"xinb{ci}_{t}", tag=f"xinb{t}", bufs=2)
                      for t in range(TPC)]
            h0T_k = [state.tile([P, CHUNK], F32R, name=f"h0T{ci}_{k}", tag=f"h0Tk{k}")
                     for k in range(KH)]
            hTa_k = [state.tile([P, CHUNK], F32R, name=f"hTa{ci}_{k}", tag=f"hTak{k}")
                     for k in range(KH)]
            hTb_k = [state.tile([P, CHUNK], F32R, name=f"hTb{ci}_{k}", tag=f"hTbk{k}")
                     for k in range(KH)]

            def transpose_into(dst_k, ti, src_bm, nm):
                """dst_k[k][:, ti*P:(ti+1)*P] <- transpose blocks of [P, H] f32r."""
                tp = psT.tile([P, H], F32R, name=f"tpk_{nm}", tag="psT")
                for k in range(KH):
                    nc.tensor.transpose(tp[:, k * P:(k + 1) * P],
                                        src_bm[:, k * P:(k + 1) * P], identr)
                for k in range(KH):
                    nc.scalar.activation(dst_k[k][:, ti * P:(ti + 1) * P],
                                         tp[:, k * P:(k + 1) * P], AF.Copy)
                return tp

            # ---- xin + t0 ----
            for ti in range(TPC):
                xin_ps = psA.tile([P, H], F32, name=f"xin_ps{ci}_{ti}", tag="psA")
                for k in range(KI):
                    nc.tensor.matmul(xin_ps, xT_sb[k][:, ti * P:(ti + 1) * P], winT_sb[k],
                                     start=(k == 0), stop=(k == KI - 1))
                if host["xrow_zero"]:
                    nc.vector.tensor_copy(out=xinb_t[ti], in_=xin_ps)
                else:
                    nc.vector.tensor_tensor(out=xinb_t[ti], in0=xin_ps, in1=xrow_b, op=OP.add)
                th = work.tile([P, H], F32, name=f"th{ci}_{ti}", tag="th")
                if host["liq0c_zero"]:
                    nc.scalar.activation(th, xinb_t[ti], AF.Tanh)
                else:
                    tmp0 = work.tile([P, H], F32, name=f"tmp0{ci}_{ti}", tag="tmp0")
                    nc.vector.tensor_tensor(out=tmp0, in0=xinb_t[ti], in1=liq0c_b, op=OP.add)
                    nc.scalar.activation(th, tmp0, AF.Tanh)
                h1 = work.tile([P, H], F32R, name=f"h1{ci}_{ti}", tag="h1")
                if c_imm is not None:
                    nc.vector.tensor_scalar_mul(out=h1, in0=th, scalar1=float(c_imm))
                else:
                    nc.vector.tensor_tensor(out=h1, in0=th, in1=cvec_row_b, op=OP.mult)
                transpose_into(h0T_k, ti, h1, f't0_{ci}_{ti}')

            # ---- steps t >= 1 (software-pipelined: the tail of tile ti
            # (transposes/att/LN/tanh/update) is emitted after the liquid
            # matmuls of tile ti+1, so the in-order PE queue always has
            # independent matmul work while DVE runs the stats chain) ----
            cur = h0T_k
            for st_i in range(1, int(steps)):
                nxt = hTa_k if (st_i % 2 == 1) else hTb_k

                def phaseA(ti, cur, st_i):
                    y_ps = psY.tile([P, L * H], F32, name=f"y{ci}_{st_i}_{ti}", tag="psY")
                    for n in range(NL):
                        for k in range(KH):
                            nc.tensor.matmul(y_ps[:, n * H:(n + 1) * H],
                                             cur[k][:, ti * P:(ti + 1) * P],
                                             wliq_sb[k][:, n * H:(n + 1) * H],
                                             start=(k == 0), stop=(k == KH - 1))
                    if not host["bliq_zero"]:
                        nc.vector.tensor_tensor(out=y_ps, in0=y_ps, in1=bliq_b, op=OP.add)
                    return y_ps

                def phaseB(ti, y_ps, cur, nxt, st_i):
                    mvL = stats.tile([P, 2, L], F32, name=f"mvL{ci}_{st_i}_{ti}", tag="mvL")
                    for l in range(L):
                        st6 = stats.tile([P, 6], F32, name=f"st6L{ci}_{st_i}_{ti}_{l}", tag="st6")
                        nc.vector.bn_stats(out=st6, in_=y_ps[:, l * H:(l + 1) * H])
                        nc.vector.bn_aggr(out=mvL[:, :, l], in_=st6)
                    RL, MBL = rstd_chain(mvL, L, f"L{ci}_{st_i}_{ti}")
                    liq = work.tile([P, H], F32R, name=f"liq{ci}_{st_i}_{ti}", tag="liq")
                    if host["g_trivial"]:
                        z0 = work.tile([P, H], F32, name=f"z0{ci}_{st_i}_{ti}", tag="th")
                        nc.scalar.activation(z0, y_ps[:, 0:H], AF.Identity,
                                             bias=MBL[:, 0:1], scale=RL[:, 0:1])
                        z1 = work.tile([P, H], F32, name=f"z1{ci}_{st_i}_{ti}", tag="h1")
                        nc.scalar.activation(z1, y_ps[:, H:2 * H], AF.Identity,
                                             bias=MBL[:, 1:2], scale=RL[:, 1:2])
                        A = work.tile([P, H], F32, name=f"A{ci}_{st_i}_{ti}", tag="A")
                        nc.vector.tensor_tensor(out=A, in0=z0, in1=z1, op=OP.add)
                        nc.vector.affine_then_add(out=liq, in0=y_ps[:, 2 * H:3 * H], in1=A,
                                                  scale=RL[:, 2:3], bias=MBL[:, 2:3])
                    else:
                        A = work.tile([P, H], F32, name=f"A{ci}_{st_i}_{ti}", tag="A")
                        for l in range(L):
                            zt = work.tile([P, H], F32, name=f"zt{ci}_{st_i}_{ti}_{l}", tag="zt")
                            nc.vector.tensor_scalar(out=zt, in0=y_ps[:, l * H:(l + 1) * H],
                                                    scalar1=RL[:, l:l + 1], scalar2=MBL[:, l:l + 1],
                                                    op0=OP.mult, op1=OP.add)
                            if l == 0:
                                nc.vector.tensor_tensor(out=A, in0=zt, in1=g_b[0], op=OP.mult)
                            else:
                                zg = work.tile([P, H], F32, name=f"zg{ci}_{st_i}_{ti}_{l}", tag="zg")
                                nc.vector.tensor_tensor(out=zg, in0=zt, in1=g_b[l], op=OP.mult)
                                nc.vector.tensor_tensor(out=(liq if l == L - 1 else A),
                                                        in0=A, in1=zg, op=OP.add)
                    liqT = work.tile([P, KH * P], F32R, name=f"liqT{ci}_{st_i}_{ti}", tag="liqT")
                    transpose_to(liqT, liq)
                    att_ps = psA.tile([P, H], F32, name=f"att{ci}_{st_i}_{ti}", tag="psA")
                    for k in range(KH):
                        nc.tensor.matmul(att_ps, liqT[:, k * P:(k + 1) * P], watt_sb[k],
                                         start=(k == 0), stop=(k == KH - 1))
                    if not host["batt_zero"]:
                        nc.vector.tensor_tensor(out=att_ps, in0=att_ps, in1=batt_b, op=OP.add)
                    mvA = stats.tile([P, 2, 1], F32, name=f"mvA{ci}_{st_i}_{ti}", tag="mvA")
                    st6a = stats.tile([P, 6], F32, name=f"st6A{ci}_{st_i}_{ti}", tag="st6")
                    nc.vector.bn_stats(out=st6a, in_=att_ps)
                    nc.vector.bn_aggr(out=mvA[:, :, 0], in_=st6a)
                    RA, MBA = rstd_chain(mvA, 1, f"A{ci}_{st_i}_{ti}")
                    z = work.tile([P, H], F32, name=f"z{ci}_{st_i}_{ti}", tag="z")
                    if host["gatt_trivial"]:
                        nc.vector.affine_then_add(out=z, in0=att_ps, in1=xinb_t[ti],
                                                  scale=RA[:, 0:1], bias=MBA[:, 0:1])
                    else:
                        zn = work.tile([P, H], F32, name=f"zn{ci}_{st_i}_{ti}", tag="zn")
                        nc.vector.tensor_scalar(out=zn, in0=att_ps, scalar1=RA[:, 0:1],
                                                scalar2=MBA[:, 0:1], op0=OP.mult, op1=OP.add)
                        nc.vector.tensor_tensor(out=zn, in0=zn, in1=gatt_b, op=OP.mult)
                        nc.vector.tensor_tensor(out=z, in0=zn, in1=xinb_t[ti], op=OP.add)
                    tt = work.tile([P, H], F32R, name=f"tt{ci}_{st_i}_{ti}", tag="tt")
                    nc.scalar.activation(tt, z, AF.Tanh)
                    tp = psT.tile([P, H], F32R, name=f"tpu{ci}_{st_i}_{ti}", tag="psT")
                    for k in range(KH):
                        nc.tensor.transpose(tp[:, k * P:(k + 1) * P],
                                            tt[:, k * P:(k + 1) * P], identr)
                    u = work.tile([P, H], F32, name=f"u{ci}_{st_i}_{ti}", tag="u")
                    if c_imm is not None:
                        nc.scalar.activation(u, tp.bitcast(F32), AF.Copy, scale=float(c_imm))
                        for k in range(KH):
                            nc.vector.scalar_tensor_tensor(
                                out=nxt[k][:, ti * P:(ti + 1) * P],
                                in0=cur[k][:, ti * P:(ti + 1) * P],
                                scalar=float(a_imm), in1=u[:, k * P:(k + 1) * P],
                                op0=OP.mult, op1=OP.add)
                    else:
                        for k in range(KH):
                            nc.scalar.activation(u[:, k * P:(k + 1) * P],
                                                 tp.bitcast(F32)[:, k * P:(k + 1) * P],
                                                 AF.Copy, scale=cvec_sb[:, k:k + 1])
                            uk = work.tile([P, P], F32, name=f"uk{ci}_{st_i}_{ti}_{k}", tag="uk")
                            nc.vector.tensor_scalar_mul(out=uk, in0=cur[k][:, ti * P:(ti + 1) * P],
                                                        scalar1=avec_sb[:, k:k + 1])
                            nc.vector.tensor_tensor(out=nxt[k][:, ti * P:(ti + 1) * P],
                                                    in0=uk, in1=u[:, k * P:(k + 1) * P], op=OP.add)

                pend = [None] * TPC
                for ti in range(TPC):
                    pend[ti] = phaseA(ti, cur, st_i)
                    if ti >= 1:
                        phaseB(ti - 1, pend[ti - 1], cur, nxt, st_i)
                phaseB(TPC - 1, pend[TPC - 1], cur, nxt, st_i)
                cur = nxt

            # ---- output stage ----
            SLAB = 512 if CHUNK % 512 == 0 else CHUNK
            for nj in range(CHUNK // SLAB):
                tlo = nj * (SLAB // P)
                o2_ps = psT.tile([P, SLAB], F32, name=f"o2{ci}_{nj}", tag="psT")
                for m in range(M1):
                    o1_ps = psA.tile([P, SLAB], F32, name=f"o1{ci}_{nj}_{m}", tag="psA")
                    for kb in range(K1):
                        hsrc = cur if kb < KH else h0T_k
                        k = kb % KH
                        nc.tensor.matmul(
                            o1_ps,
                            w1T_sb[kb][:, m * P:(m + 1) * P],
                            hsrc[k][:, nj * SLAB:(nj + 1) * SLAB],
                            start=(kb == 0), stop=(kb == K1 - 1))
                    rlu = work.tile([P, SLAB], F32R, name=f"rlu{ci}_{nj}_{m}", tag="rlu")
                    if host["b1_zero"]:
                        nc.scalar.activation(rlu, o1_ps, AF.Relu)
                    else:
                        nc.scalar.activation(rlu, o1_ps, AF.Relu, bias=b1_sb[:, m:m + 1])
                    nc.tensor.matmul(o2_ps, w2T_sb[m], rlu, start=(m == 0), stop=(m == M1 - 1))
                o_sb = work.tile([P, SLAB], F32, name=f"osb{ci}_{nj}", tag="osb")
                if host["b2_zero"]:
                    nc.scalar.activation(o_sb, o2_ps, AF.Copy)
                else:
                    nc.scalar.activation(o_sb, o2_ps, AF.Identity, bias=b2_sb[:, 0:1])
                nc.sync.dma_start(out=out_t[0:OUT, c0 + nj * SLAB:c0 + (nj + 1) * SLAB],
                                  in_=o_sb[0:OUT, :])
    nc.finalize()
    return nc


def _prep_host(inputs):
    x = np.asarray(inputs["x"], np.float32)
    W_in = np.asarray(inputs["W_in"], np.float32)
    b_in = np.asarray(inputs["b_in"], np.float32)
    W_liq = np.asarray(inputs["W_liq"], np.float32)
    b_liq = np.asarray(inputs["b_liq"], np.float32)
    g_liq = np.asarray(inputs["g_liq"], np.float32)
    beta_liq = np.asarray(inputs["beta_liq"], np.float32)
    W_v = np.asarray(inputs["W_v"], np.float32)
    b_v = np.asarray(inputs["b_v"], np.float32)
    W_o = np.asarray(inputs["W_o"], np.float32)
    b_o = np.asarray(inputs["b_o"], np.float32)
    g_att = np.asarray(inputs["g_att"], np.float32)
    beta_att = np.asarray(inputs["beta_att"], np.float32)
    tau = np.asarray(inputs["tau"], np.float32)
    W1 = np.asarray(inputs["W1"], np.float32)
    b1 = np.asarray(inputs["b1"], np.float32)
    W2 = np.asarray(inputs["W2"], np.float32)
    b2 = np.asarray(inputs["b2"], np.float32)
    steps = int(np.asarray(inputs["steps"]))

    B, IN = x.shape
    L, H, _ = W_liq.shape
    OUT = W2.shape[0]
    I = np.eye(H, dtype=np.float32)
    Wl = (W_liq + I).astype(np.float32)
    W_att = (W_o @ W_v + I).astype(np.float32)
    Bsum = beta_liq.sum(0).astype(np.float32)
    b_att = (W_att @ Bsum + W_o @ b_v + b_o).astype(np.float32)
    c_vec = (1.0 / (tau + TAU_EPS)).astype(np.float32)
    a_vec = (1.0 - c_vec).astype(np.float32)
    liq0 = sum(_np_ln(b_liq[l]) * g_liq[l] + beta_liq[l] for l in range(L))
    att0 = (liq0 @ W_v.T + b_v) @ W_o.T + b_o
    liq0c = (_np_ln(att0 + liq0) * g_att + beta_att).astype(np.float32)
    # on-chip z = LN(att)+xinb with xinb = xin + (b_in + beta_att)
    xrow = (b_in + beta_att).astype(np.float32)
    # t0 wants tanh(xin + liq0c) = tanh(xinb + (liq0c - b_in - beta_att))
    liq0c_eff = (liq0c - xrow).astype(np.float32)

    wliq_mov = np.empty((H, L * H), np.float32)
    for l in range(L):
        wliq_mov[:, l * H:(l + 1) * H] = Wl[l].T
    host = {
        "B": B, "IN": IN, "H": H, "OUT": OUT, "L": L, "steps": steps,
        "c_imm": float(c_vec[0]) if np.all(c_vec == c_vec[0]) else None,
        "a_imm": float(a_vec[0]) if np.all(a_vec == a_vec[0]) else None,
        "g_trivial": bool(np.all(g_liq == 1.0)),
        "gatt_trivial": bool(np.all(g_att == 1.0)),
        "liq0c_zero": bool(np.all(liq0c_eff == 0.0)),
        "xrow_zero": bool(np.all(xrow == 0.0)),
        "b1_zero": bool(np.all(b1 == 0.0)),
        "b2_zero": bool(np.all(b2 == 0.0)),
        "bliq_zero": bool(np.all(b_liq == 0.0)),
        "batt_zero": bool(np.all(b_att == 0.0)),
    }
    rows = np.zeros((8, H), np.float32)
    rows[0] = liq0c_eff
    rows[1] = xrow
    rows[2] = c_vec
    rows[3] = a_vec
    rows[4:4 + L] = g_liq
    rows[7] = g_att
    bliqrow = np.empty((1, L * H), np.float32)
    for l in range(L):
        bliqrow[0, l * H:(l + 1) * H] = b_liq[l]
    tensors = {
        "wliq": np.ascontiguousarray(wliq_mov),
        "watt": np.ascontiguousarray(W_att.T),
        "winT": np.ascontiguousarray(W_in.T),
        "w1T": np.ascontiguousarray(W1.T),
        "w2T": np.ascontiguousarray(W2.T),
        "rows": rows,
        "bliqrow": bliqrow,
        "battrow": b_att.reshape(1, H).astype(np.float32),
        "b1row": b1.reshape(1, H).astype(np.float32),
        "b2row": b2.reshape(1, OUT).astype(np.float32),
    }
    return x, host, tensors


_NC_CACHE = {}


def kernel(**inputs) -> np.ndarray:
    x, host, tensors = _prep_host(inputs)
    B, H, IN, OUT, L = host["B"], host["H"], host["IN"], host["OUT"], host["L"]
    steps = host["steps"]
    B_shard = B // N_CORES
    CHUNK = 1024 if B_shard % 1024 == 0 else B_shard
    key = (B_shard, H, IN, OUT, L, steps, CHUNK,
           tuple(sorted((k, str(v)) for k, v in host.items())))
    if key not in _NC_CACHE:
        _NC_CACHE[key] = build_nc(B_shard, H, IN, OUT, L, steps, CHUNK, host)
    nc = _NC_CACHE[key]

    xT = np.ascontiguousarray(x.T)
    in_maps = []
    for c in range(N_CORES):
        m = dict(tensors)
        m["xT"] = np.ascontiguousarray(xT[:, c * B_shard:(c + 1) * B_shard])
        in_maps.append(m)
    res = run_bass_kernel_spmd(nc, in_maps, core_ids=list(range(N_CORES)))
    outs = [res.results[c]["out_t"] for c in range(N_CORES)]
    full = np.concatenate(outs, axis=1)
    return np.ascontiguousarray(full.T).astype(np.float32)



# revision 5
# speedup vs baseline: 1.6045x; 1.6045x over previous
"""Trainium2 Bass kernel for nn_AdvancedLiquidNeuralNetwork.

Data-parallel over 8 NeuronCores (batch sharded). Per core, batch-major
compute with a feature-major f32r s-state (s = h / c, c = 1/(tau+eps)
absorbed into the liquid / output weights so the state update is
s' = a*s + tanh(z) with no extra scaling pass):

  xin = x @ W_in.T                   (PE, x.T-stationary -> batch-major PSUM)
  t=0:  s1 = tanh(xin [+ liq0c])     (Scalar, straight from PSUM)
  t>=1: y_l = s @ (c*(W_liq_l+I)).T  (PE, sT-stationary, per-layer PSUM tiles)
        LN stats: bn_stats/bn_aggr;  rstd = recip_approx(Scalar sqrt(var+eps))
        liq = y2*R2 + (y1*R1 + (y0*R0 + sum MB))   (1 Scalar + 2 DVE stt)
        att  = liq @ (W_o W_v + I).T (PE, liqT-stationary)
        z'   = LN(att) (Scalar), z = z' + xin (DVE bf16), t = tanh(z) (Scalar)
        s'T  = a*sT + tT             (single strided DVE stt per tile)
  out = relu([s3 s0] @ (c*W1).T + b1) @ W2.T + b2  (feature-major, fused ACT)

Three-stage software pipeline per step (liquid-mm | LN+att | z+update) so
PE / DVE / Scalar overlap; PSUM split 4+2+2 banks (y / att / transpose).
"""
import sys
import numpy as np
from contextlib import ExitStack

sys.path.insert(0, "/opt/trn_rl_repo")

import concourse.bacc as bacc
import concourse.bass as bass
import concourse.tile as tile
from concourse import mybir
from concourse.bass_utils import run_bass_kernel_spmd
from concourse.masks import make_identity

F32 = mybir.dt.float32
F32R = mybir.dt.float32r
BF16 = mybir.dt.bfloat16
AF = mybir.ActivationFunctionType
OP = mybir.AluOpType

LN_EPS = 1e-5
TAU_EPS = 1e-6
N_CORES = 8
P = 128


def _np_ln(x, eps=LN_EPS):
    m = x.mean(-1, keepdims=True)
    v = x.var(-1, keepdims=True)
    return (x - m) / np.sqrt(v + eps)


def build_nc(B_shard, H, IN, OUT, L, steps, CHUNK, host):
    nc = bacc.Bacc("TRN2")
    n_chunks = B_shard // CHUNK
    TPC = CHUNK // P       # tiles per chunk
    KH = H // P            # 4
    KI = IN // P           # 2
    M1 = H // P            # 4 m-tiles of out1
    K1 = 2 * H // P        # 8 k-tiles of out1
    a_imm = host["a_imm"]

    xT = nc.dram_tensor("xT", [IN, B_shard], F32, kind="ExternalInput")
    wliq = nc.dram_tensor("wliq", [H, L * H], F32, kind="ExternalInput")
    watt = nc.dram_tensor("watt", [H, H], F32, kind="ExternalInput")
    winT = nc.dram_tensor("winT", [IN, H], F32, kind="ExternalInput")
    w1T = nc.dram_tensor("w1T", [2 * H, H], F32, kind="ExternalInput")
    w2T = nc.dram_tensor("w2T", [H, OUT], F32, kind="ExternalInput")
    rows = nc.dram_tensor("rows", [8, H], F32, kind="ExternalInput")
    bliqrow = nc.dram_tensor("bliqrow", [1, L * H], F32, kind="ExternalInput")
    battrow = nc.dram_tensor("battrow", [1, H], F32, kind="ExternalInput")
    b1row = nc.dram_tensor("b1row", [1, H], F32, kind="ExternalInput")
    b2row = nc.dram_tensor("b2row", [1, OUT], F32, kind="ExternalInput")
    out_t = nc.dram_tensor("out_t", [OUT, B_shard], F32, kind="ExternalOutput")

    with tile.TileContext(nc) as tc, ExitStack() as ctx:
        wpool = ctx.enter_context(tc.tile_pool(name="wpool", bufs=1))
        stage = ctx.enter_context(tc.tile_pool(name="stage", bufs=1))
        state = ctx.enter_context(tc.tile_pool(name="state", bufs=1))
        work = ctx.enter_context(tc.tile_pool(name="work", bufs=2))
        stats = ctx.enter_context(tc.tile_pool(name="stats", bufs=3))
        psY = ctx.enter_context(tc.tile_pool(name="psY", bufs=4, space="PSUM"))
        psA = ctx.enter_context(tc.tile_pool(name="psA", bufs=2, space="PSUM"))
        psT = ctx.enter_context(tc.tile_pool(name="psT", bufs=2, space="PSUM"))

        def load_round(dram, nrow, ncol, name):
            outs = []
            for k in range(nrow // P):
                st = stage.tile([P, ncol], F32, name=f"st_{name}{k}", tag="stage")
                t = wpool.tile([P, ncol], F32R, name=f"{name}{k}")
                nc.sync.dma_start(out=st, in_=dram[k * P:(k + 1) * P, :])
                nc.scalar.activation(t, st, AF.Copy)
                outs.append(t)
            return outs

        wliq_sb = load_round(wliq, H, L * H, "wliq")
        watt_sb = load_round(watt, H, H, "watt")
        winT_sb = load_round(winT, IN, H, "winT")
        w1T_sb = load_round(w1T, 2 * H, H, "w1T")
        w2T_sb = load_round(w2T, H, OUT, "w2T")

        ident = wpool.tile([P, P], F32, name="ident")
        make_identity(nc, ident)
        identr = wpool.tile([P, P], F32R, name="identr")
        nc.scalar.activation(identr, ident, AF.Copy)

        epsC = wpool.tile([P, 1], F32, name="epsC")
        nc.vector.memset(epsC, LN_EPS)

        def bcast_row(row_ap, n, name, dt=F32):
            t = wpool.tile([P, n], dt, name=name)
            src = bass.AP(tensor=row_ap.tensor, offset=row_ap.offset, ap=[[0, P], [1, n]])
            if dt == F32:
                nc.sync.dma_start(out=t, in_=src)
            else:
                st = stage.tile([P, n], F32, name=f"st_{name}", tag="stage")
                nc.sync.dma_start(out=st, in_=src)
                nc.scalar.activation(t, st, AF.Copy)
            return t

        liq0c_b = None if host["liq0c_zero"] else bcast_row(rows[0:1, :], H, "liq0c_b")
        xrow_b = None if host["xrow_zero"] else bcast_row(rows[1:2, :], H, "xrow_b")
        g_b = None if host["g_trivial"] else [bcast_row(rows[4 + l:5 + l, :], H, f"g_b{l}") for l in range(L)]
        gatt_b = None if host["gatt_trivial"] else bcast_row(rows[7:8, :], H, "gatt_b")
        bliq_b = None if host["bliq_zero"] else bcast_row(bliqrow[0:1, :], L * H, "bliq_b")
        batt_b = None if host["batt_zero"] else bcast_row(battrow[0:1, :], H, "batt_b")
        avec_sb = None
        if a_imm is None:
            avec_sb = wpool.tile([P, KH], F32, name="avec_sb")
            nc.sync.dma_start(out=avec_sb, in_=bass.AP(
                tensor=rows.tensor, offset=rows[3:4, :].offset, ap=[[1, P], [P, KH]]))
        b1_sb = None
        if not host["b1_zero"]:
            b1_sb = wpool.tile([P, M1], F32, name="b1_sb")
            nc.sync.dma_start(out=b1_sb, in_=bass.AP(
                tensor=b1row.tensor, offset=0, ap=[[1, P], [P, M1]]))
        b2_sb = None
        if not host["b2_zero"]:
            b2_sb = wpool.tile([P, 1], F32, name="b2_sb")
            nc.sync.dma_start(out=b2_sb, in_=bass.AP(
                tensor=b2row.tensor, offset=0, ap=[[1, P], [P, 1]]))

        for ci in range(n_chunks):
            c0 = ci * CHUNK
            xT_sb = []
            for k in range(KI):
                xst = stage.tile([P, CHUNK], F32, name=f"xst{ci}_{k}", tag="xstage", bufs=2)
                xsb = state.tile([P, CHUNK], F32R, name=f"xTs{ci}_{k}", tag=f"xT{k}", bufs=2)
                nc.sync.dma_start(out=xst, in_=xT[k * P:(k + 1) * P, c0:c0 + CHUNK])
                nc.scalar.activation(xsb, xst, AF.Copy)
                xT_sb.append(xsb)

            # batch-major loop-invariant input drive, bf16 (feeds the z add)
            xinb = state.tile([P, TPC, H], BF16, name=f"xinb{ci}", tag="xinb", bufs=2)
            # feature-major states [P, KH, CHUNK]
            h0T = state.tile([P, KH, CHUNK], F32R, name=f"h0T{ci}", tag="h0T", bufs=2)
            hTa = state.tile([P, KH, CHUNK], F32R, name=f"hTa{ci}", tag="hTa")
            hTb = state.tile([P, KH, CHUNK], F32R, name=f"hTb{ci}", tag="hTb")

            # ---- xin + t0 ----
            for ti in range(TPC):
                xin_ps = psA.tile([P, H], F32, name=f"xin_ps{ci}_{ti}", tag="psA")
                for k in range(KI):
                    nc.tensor.matmul(xin_ps, xT_sb[k][:, ti * P:(ti + 1) * P], winT_sb[k],
                                     start=(k == 0), stop=(k == KI - 1))
                if host["xrow_zero"]:
                    nc.scalar.activation(xinb[:, ti, :], xin_ps, AF.Copy)
                else:
                    nc.vector.tensor_tensor(out=xinb[:, ti, :], in0=xin_ps, in1=xrow_b, op=OP.add)
                th = work.tile([P, H], F32R, name=f"th{ci}_{ti}", tag="th")
                if host["liq0c_zero"]:
                    nc.scalar.activation(th, xin_ps, AF.Tanh)
                    if not host["xrow_zero"]:
                        # th currently tanh(xin) w/o xrow: recompute via vector path
                        tmp0 = work.tile([P, H], F32, name=f"tmp0{ci}_{ti}", tag="tmp0")
                        nc.vector.tensor_tensor(out=tmp0, in0=xin_ps, in1=xrow_b, op=OP.add)
                        nc.scalar.activation(th, tmp0, AF.Tanh)
                else:
                    tmp0 = work.tile([P, H], F32, name=f"tmp0{ci}_{ti}", tag="tmp0")
                    nc.vector.tensor_tensor(out=tmp0, in0=xin_ps, in1=liq0c_b, op=OP.add)
                    if not host["xrow_zero"]:
                        nc.vector.tensor_tensor(out=tmp0, in0=tmp0, in1=xrow_b, op=OP.add)
                    nc.scalar.activation(th, tmp0, AF.Tanh)
                tp0 = psT.tile([P, KH, P], F32R, name=f"tp0{ci}_{ti}", tag="psT")
                for k in range(KH):
                    nc.tensor.transpose(tp0[:, k, :], th[:, k * P:(k + 1) * P], identr)
                nc.scalar.activation(h0T[:, :, ti * P:(ti + 1) * P], tp0, AF.Copy)

            # ---- steps t >= 1: 3-stage pipeline ----
            cur = h0T
            for st_i in range(1, int(steps)):
                nxt = hTa if (st_i % 2 == 1) else hTb

                def stageA(ti):
                    ys = []
                    for n in range(L):
                        y = psY.tile([P, H], F32, name=f"y{ci}_{st_i}_{ti}_{n}", tag="psY")
                        for k in range(KH):
                            nc.tensor.matmul(y, cur[:, k, ti * P:(ti + 1) * P],
                                             wliq_sb[k][:, n * H:(n + 1) * H],
                                             start=(k == 0), stop=(k == KH - 1))
                        if not host["bliq_zero"]:
                            nc.vector.tensor_tensor(out=y, in0=y,
                                                    in1=bliq_b[:, n * H:(n + 1) * H], op=OP.add)
                        ys.append(y)
                    return ys

                def stageB(ti, ys):
                    mvL = stats.tile([P, 2, L], F32, name=f"mvL{ci}_{st_i}_{ti}", tag="mvL")
                    for l in range(L):
                        st6 = stats.tile([P, 6], F32, name=f"st6L{ci}_{st_i}_{ti}_{l}", tag="st6")
                        nc.vector.bn_stats(out=st6, in_=ys[l])
                        nc.vector.bn_aggr(out=mvL[:, :, l], in_=st6)
                    sq = stats.tile([P, L], F32, name=f"sq{ci}_{st_i}_{ti}", tag="sq")
                    nc.scalar.activation(sq, mvL[:, 1, :], AF.Sqrt, bias=epsC[:, 0:1])
                    R = stats.tile([P, L], F32, name=f"R{ci}_{st_i}_{ti}", tag="R")
                    nc.vector.reciprocal_approx_fast(out=R, in_=sq)
                    MB = stats.tile([P, L], F32, name=f"MB{ci}_{st_i}_{ti}", tag="MB")
                    nc.vector.scalar_tensor_tensor(out=MB, in0=mvL[:, 0, :], scalar=-1.0,
                                                   in1=R, op0=OP.mult, op1=OP.mult)
                    liq = work.tile([P, H], F32R, name=f"liq{ci}_{st_i}_{ti}", tag="liq")
                    if host["g_trivial"]:
                        MBs = stats.tile([P, 1], F32, name=f"MBs{ci}_{st_i}_{ti}", tag="MBs")
                        nc.vector.tensor_reduce(out=MBs, in_=MB, axis=mybir.AxisListType.X,
                                                op=OP.add)
                        z0 = work.tile([P, H], F32, name=f"z0{ci}_{st_i}_{ti}", tag="z0")
                        nc.scalar.activation(z0, ys[0], AF.Identity,
                                             bias=MBs[:, 0:1], scale=R[:, 0:1])
                        A2 = work.tile([P, H], F32, name=f"A2{ci}_{st_i}_{ti}", tag="A2")
                        nc.vector.scalar_tensor_tensor(out=A2, in0=ys[1], scalar=R[:, 1:2],
                                                       in1=z0, op0=OP.mult, op1=OP.add)
                        nc.vector.scalar_tensor_tensor(out=liq, in0=ys[2], scalar=R[:, 2:3],
                                                       in1=A2, op0=OP.mult, op1=OP.add)
                    else:
                        A2 = work.tile([P, H], F32, name=f"A2{ci}_{st_i}_{ti}", tag="A2")
                        for l in range(L):
                            zt = work.tile([P, H], F32, name=f"zt{ci}_{st_i}_{ti}_{l}", tag="zt")
                            nc.vector.tensor_scalar(out=zt, in0=ys[l],
                                                    scalar1=R[:, l:l + 1], scalar2=MB[:, l:l + 1],
                                                    op0=OP.mult, op1=OP.add)
                            if l == 0:
                                nc.vector.tensor_tensor(out=A2, in0=zt, in1=g_b[0], op=OP.mult)
                            else:
                                zg = work.tile([P, H], F32, name=f"zg{ci}_{st_i}_{ti}_{l}", tag="zg")
                                nc.vector.tensor_tensor(out=zg, in0=zt, in1=g_b[l], op=OP.mult)
                                nc.vector.tensor_tensor(out=(liq if l == L - 1 else A2),
                                                        in0=A2, in1=zg, op=OP.add)
                    tpl = psT.tile([P, KH, P], F32R, name=f"tpl{ci}_{st_i}_{ti}", tag="psT")
                    for k in range(KH):
                        nc.tensor.transpose(tpl[:, k, :], liq[:, k * P:(k + 1) * P], identr)
                    liqT = work.tile([P, KH, P], F32R, name=f"liqT{ci}_{st_i}_{ti}", tag="liqT")
                    nc.scalar.activation(liqT, tpl, AF.Copy)
                    att_ps = psA.tile([P, H], F32, name=f"att{ci}_{st_i}_{ti}", tag="psA")
                    for k in range(KH):
                        nc.tensor.matmul(att_ps, liqT[:, k, :], watt_sb[k],
                                         start=(k == 0), stop=(k == KH - 1))
                    if not host["batt_zero"]:
                        nc.vector.tensor_tensor(out=att_ps, in0=att_ps, in1=batt_b, op=OP.add)
                    return att_ps

                def stageC(ti, att_ps):
                    st6a = stats.tile([P, 6], F32, name=f"st6A{ci}_{st_i}_{ti}", tag="st6a")
                    nc.vector.bn_stats(out=st6a, in_=att_ps)
                    mvA = stats.tile([P, 2], F32, name=f"mvA{ci}_{st_i}_{ti}", tag="mvA")
                    nc.vector.bn_aggr(out=mvA, in_=st6a)
                    sqA = stats.tile([P, 1], F32, name=f"sqA{ci}_{st_i}_{ti}", tag="sqA")
                    nc.scalar.activation(sqA, mvA[:, 1:2], AF.Sqrt, bias=epsC[:, 0:1])
                    RA = stats.tile([P, 1], F32, name=f"RA{ci}_{st_i}_{ti}", tag="RA")
                    nc.vector.reciprocal_approx_fast(out=RA, in_=sqA)
                    MBA = stats.tile([P, 1], F32, name=f"MBA{ci}_{st_i}_{ti}", tag="MBA")
                    nc.vector.scalar_tensor_tensor(out=MBA, in0=mvA[:, 0:1], scalar=-1.0,
                                                   in1=RA, op0=OP.mult, op1=OP.mult)
                    zp = work.tile([P, H], BF16, name=f"zp{ci}_{st_i}_{ti}", tag="zp")
                    nc.scalar.activation(zp, att_ps, AF.Identity,
                                         bias=MBA[:, 0:1], scale=RA[:, 0:1])
                    if not host["gatt_trivial"]:
                        nc.vector.tensor_tensor(out=zp, in0=zp, in1=gatt_b, op=OP.mult)
                    z = work.tile([P, H], BF16, name=f"z{ci}_{st_i}_{ti}", tag="z")
                    nc.vector.tensor_tensor(out=z, in0=zp, in1=xinb[:, ti, :], op=OP.add)
                    tt = work.tile([P, H], F32R, name=f"tt{ci}_{st_i}_{ti}", tag="tt")
                    nc.scalar.activation(tt, z, AF.Tanh)
                    tpu = psT.tile([P, KH, P], F32R, name=f"tpu{ci}_{st_i}_{ti}", tag="psT")
                    for k in range(KH):
                        nc.tensor.transpose(tpu[:, k, :], tt[:, k * P:(k + 1) * P], identr)
                    if a_imm is not None:
                        nc.vector.scalar_tensor_tensor(
                            out=nxt[:, :, ti * P:(ti + 1) * P],
                            in0=cur[:, :, ti * P:(ti + 1) * P],
                            scalar=float(a_imm), in1=tpu, op0=OP.mult, op1=OP.add)
                    else:
                        for k in range(KH):
                            nc.vector.scalar_tensor_tensor(
                                out=nxt[:, k, ti * P:(ti + 1) * P],
                                in0=cur[:, k, ti * P:(ti + 1) * P],
                                scalar=avec_sb[:, k:k + 1], in1=tpu[:, k, :],
                                op0=OP.mult, op1=OP.add)

                pendA = [None] * TPC
                pendB = [None] * TPC
                for ti in range(TPC):
                    pendA[ti] = stageA(ti)
                    if ti >= 1:
                        pendB[ti - 1] = stageB(ti - 1, pendA[ti - 1])
                    if ti >= 2:
                        stageC(ti - 2, pendB[ti - 2])
                pendB[TPC - 1] = stageB(TPC - 1, pendA[TPC - 1])
                stageC(TPC - 2, pendB[TPC - 2])
                stageC(TPC - 1, pendB[TPC - 1])
                cur = nxt

            # ---- output stage ----
            SLAB = 512 if CHUNK % 512 == 0 else CHUNK
            for nj in range(CHUNK // SLAB):
                o2_ps = psT.tile([P, SLAB], F32, name=f"o2{ci}_{nj}", tag="psT")
                for m in range(M1):
                    o1_ps = psY.tile([P, SLAB], F32, name=f"o1{ci}_{nj}_{m}", tag="psY")
                    for kb in range(K1):
                        hsrc = cur if kb < KH else h0T
                        k = kb % KH
                        nc.tensor.matmul(
                            o1_ps,
                            w1T_sb[kb][:, m * P:(m + 1) * P],
                            hsrc[:, k, nj * SLAB:(nj + 1) * SLAB],
                            start=(kb == 0), stop=(kb == K1 - 1))
                    rlu = work.tile([P, SLAB], F32R, name=f"rlu{ci}_{nj}_{m}", tag="rlu")
                    if host["b1_zero"]:
                        nc.scalar.activation(rlu, o1_ps, AF.Relu)
                    else:
                        nc.scalar.activation(rlu, o1_ps, AF.Relu, bias=b1_sb[:, m:m + 1])
                    nc.tensor.matmul(o2_ps, w2T_sb[m], rlu, start=(m == 0), stop=(m == M1 - 1))
                o_sb = work.tile([P, SLAB], F32, name=f"osb{ci}_{nj}", tag="osb")
                if host["b2_zero"]:
                    nc.scalar.activation(o_sb, o2_ps, AF.Copy)
                else:
                    nc.scalar.activation(o_sb, o2_ps, AF.Identity, bias=b2_sb[:, 0:1])
                nc.sync.dma_start(out=out_t[0:OUT, c0 + nj * SLAB:c0 + (nj + 1) * SLAB],
                                  in_=o_sb[0:OUT, :])
    nc.finalize()
    return nc


def _prep_host(inputs):
    x = np.asarray(inputs["x"], np.float32)
    W_in = np.asarray(inputs["W_in"], np.float32)
    b_in = np.asarray(inputs["b_in"], np.float32)
    W_liq = np.asarray(inputs["W_liq"], np.float32)
    b_liq = np.asarray(inputs["b_liq"], np.float32)
    g_liq = np.asarray(inputs["g_liq"], np.float32)
    beta_liq = np.asarray(inputs["beta_liq"], np.float32)
    W_v = np.asarray(inputs["W_v"], np.float32)
    b_v = np.asarray(inputs["b_v"], np.float32)
    W_o = np.asarray(inputs["W_o"], np.float32)
    b_o = np.asarray(inputs["b_o"], np.float32)
    g_att = np.asarray(inputs["g_att"], np.float32)
    beta_att = np.asarray(inputs["beta_att"], np.float32)
    tau = np.asarray(inputs["tau"], np.float32)
    W1 = np.asarray(inputs["W1"], np.float32)
    b1 = np.asarray(inputs["b1"], np.float32)
    W2 = np.asarray(inputs["W2"], np.float32)
    b2 = np.asarray(inputs["b2"], np.float32)
    steps = int(np.asarray(inputs["steps"]))

    B, IN = x.shape
    L, H, _ = W_liq.shape
    OUT = W2.shape[0]
    I = np.eye(H, dtype=np.float32)
    c_vec = (1.0 / (tau + TAU_EPS)).astype(np.float32)
    a_vec = (1.0 - c_vec).astype(np.float32)
    Wl = (W_liq + I).astype(np.float32)
    W_att = (W_o @ W_v + I).astype(np.float32)
    Bsum = beta_liq.sum(0).astype(np.float32)
    b_att = (W_att @ Bsum + W_o @ b_v + b_o).astype(np.float32)
    liq0 = sum(_np_ln(b_liq[l]) * g_liq[l] + beta_liq[l] for l in range(L))
    att0 = (liq0 @ W_v.T + b_v) @ W_o.T + b_o
    liq0c = (_np_ln(att0 + liq0) * g_att + beta_att).astype(np.float32)
    # on-chip z = LN(att)+xinb with xinb = xin + (b_in + beta_att)
    xrow = (b_in + beta_att).astype(np.float32)
    # t0 wants tanh(xin + liq0c) = tanh(xinb + (liq0c - b_in - beta_att))
    liq0c_eff = (liq0c - xrow).astype(np.float32)

    # s-reparam: state s = h / c ; absorb c into liquid + output weights
    wliq_mov = np.empty((H, L * H), np.float32)
    for l in range(L):
        wliq_mov[:, l * H:(l + 1) * H] = Wl[l].T * c_vec[:, None]
    w1T = np.ascontiguousarray(W1.T) * np.concatenate([c_vec, c_vec])[:, None]
    host = {
        "B": B, "IN": IN, "H": H, "OUT": OUT, "L": L, "steps": steps,
        "a_imm": float(a_vec[0]) if np.all(a_vec == a_vec[0]) else None,
        "g_trivial": bool(np.all(g_liq == 1.0)),
        "gatt_trivial": bool(np.all(g_att == 1.0)),
        "liq0c_zero": bool(np.all(liq0c_eff == 0.0)),
        "xrow_zero": bool(np.all(xrow == 0.0)),
        "b1_zero": bool(np.all(b1 == 0.0)),
        "b2_zero": bool(np.all(b2 == 0.0)),
        "bliq_zero": bool(np.all(b_liq == 0.0)),
        "batt_zero": bool(np.all(b_att == 0.0)),
    }
    rows = np.zeros((8, H), np.float32)
    rows[0] = liq0c_eff
    rows[1] = xrow
    rows[2] = c_vec
    rows[3] = a_vec
    rows[4:4 + L] = g_liq
    rows[7] = g_att
    bliqrow = np.empty((1, L * H), np.float32)
    for l in range(L):
        bliqrow[0, l * H:(l + 1) * H] = b_liq[l]
    tensors = {
        "wliq": np.ascontiguousarray(wliq_mov),
        "watt": np.ascontiguousarray(W_att.T),
        "winT": np.ascontiguousarray(W_in.T),
        "w1T": np.ascontiguousarray(w1T.astype(np.float32)),
        "w2T": np.ascontiguousarray(W2.T),
        "rows": rows,
        "bliqrow": bliqrow,
        "battrow": b_att.reshape(1, H).astype(np.float32),
        "b1row": b1.reshape(1, H).astype(np.float32),
        "b2row": b2.reshape(1, OUT).astype(np.float32),
    }
    return x, host, tensors


_NC_CACHE = {}


def kernel(**inputs) -> np.ndarray:
    x, host, tensors = _prep_host(inputs)
    B, H, IN, OUT, L = host["B"], host["H"], host["IN"], host["OUT"], host["L"]
    steps = host["steps"]
    B_shard = B // N_CORES
    CHUNK = 1024 if B_shard % 1024 == 0 else B_shard
    key = (B_shard, H, IN, OUT, L, steps, CHUNK,
           tuple(sorted((k, str(v)) for k, v in host.items())))
    if key not in _NC_CACHE:
        _NC_CACHE[key] = build_nc(B_shard, H, IN, OUT, L, steps, CHUNK, host)
    nc = _NC_CACHE[key]

    xT = np.ascontiguousarray(x.T)
    in_maps = []
    for c in range(N_CORES):
        m = dict(tensors)
        m["xT"] = np.ascontiguousarray(xT[:, c * B_shard:(c + 1) * B_shard])
        in_maps.append(m)
    res = run_bass_kernel_spmd(nc, in_maps, core_ids=list(range(N_CORES)))
    outs = [res.results[c]["out_t"] for c in range(N_CORES)]
    full = np.concatenate(outs, axis=1)
    return np.ascontiguousarray(full.T).astype(np.float32)
